# revision 19
# baseline (speedup 1.0000x reference)
"""Hawk RG-LRU block kernel for Trainium2, 8-core SPMD.

Sharding: (batch n, time-half) -> 8 shards of [T/2=2048, ...] each.
Zero cross-core communication: second-half cores recompute a W=128-step
warmup window before their half; the RG-LRU decay (alpha ~ exp(-8*sp*sig))
makes the true carry influence < 1e-10 after 128 steps for this data regime.
First-half cores run the same program with the warmup scan input masked to
zero (wmask=0), so h is exactly 0 entering t=0.

Per core, three phases through DRAM scratch:
  A: x -> (PE transpose) -> gx = W_in @ x -> gelu(gate) -> gate_s (bf16)
                                  \-> depthwise causal conv -> xb_s (bf16)
  B: xb_s -> fg = W_g @ xb -> sigmoid/exp/sqrt gates -> tensor_tensor_scan
             -> h (fp32); z = gelu_gate * h -> z_s (bf16)
  C: z_s -> out = W_out @ z -> (PE transpose) -> out[t, d] (fp32)

Matmul operands are bf16 (weights prepacked on host); accumulation is fp32
in PSUM. The recurrence (alpha, xs, h) stays fp32 on the Vector engine via
the native tensor_tensor_scan instruction. Activation-engine work is grouped
by function (sigmoid x24, exp x12, sqrt x12 per tile) to avoid ACT LUT
reload thrash.
"""

import numpy as np
import ml_dtypes

import concourse.bass as bass
import concourse.tile as tile
from concourse import bacc, mybir
from concourse.bass_utils import run_bass_kernel_spmd

F32 = mybir.dt.float32
BF16 = mybir.dt.bfloat16
AF = mybir.ActivationFunctionType
ALU = mybir.AluOpType

EPS = 1e-6


def build_nc(T_loc=2048, W=128, TBA=512, TBB=512, TBC=512, D=1024, H=1536):
    """Build the per-core program. All 8 cores run this same program."""
    TE = W + T_loc
    nD = D // 128     # d-blocks (8)
    nH = H // 128     # h-blocks (12)
    nG = 2 * nH       # fg output blocks (24)
    assert T_loc % TBA == 0 and T_loc % TBB == 0 and T_loc % TBC == 0

    nc = bacc.Bacc("TRN2", target_bir_lowering=False, debug=False)

    # ---- external I/O ----
    xin_d = nc.dram_tensor("xin", [TE, D], F32, kind="ExternalInput")
    winT_d = nc.dram_tensor("winT", [D, 2 * H], BF16, kind="ExternalInput")
    wgT_d = nc.dram_tensor("wgT", [H, 2 * H], BF16, kind="ExternalInput")
    woutT_d = nc.dram_tensor("woutT", [H, D], BF16, kind="ExternalInput")
    cw_d = nc.dram_tensor("cw", [H, 4], F32, kind="ExternalInput")
    cb_d = nc.dram_tensor("cb", [H], F32, kind="ExternalInput")
    cvec_d = nc.dram_tensor("cvec", [H], F32, kind="ExternalInput")
    bg_d = nc.dram_tensor("bg", [2 * H], F32, kind="ExternalInput")
    wmask_d = nc.dram_tensor("wmask", [128], F32, kind="ExternalInput")
    ident_d = nc.dram_tensor("ident", [128, 128], F32, kind="ExternalInput")
    out_d = nc.dram_tensor("out", [T_loc, D], F32, kind="ExternalOutput")

    # ---- DRAM scratch ----
    xb_s = nc.dram_tensor("xb_s", [nH, 128, TE], BF16)
    gate_s = nc.dram_tensor("gate_s", [nH, 128, T_loc], BF16)
    z_s = nc.dram_tensor("z_s", [nH, 128, T_loc], BF16)

    def tiles_of(tb):
        out = [(0, W, True)]
        out += [(W + k * tb, tb, False) for k in range(T_loc // tb)]
        return out

    with tile.TileContext(nc) as tc:
        with tc.tile_pool(name="consts", bufs=1) as consts:
            ident = consts.tile([128, 128], F32, tag="ident")
            nc.sync.dma_start(ident[:], ident_d[:, :])
            cw_sb = consts.tile([128, nH, 4], F32, tag="cw")
            nc.sync.dma_start(
                cw_sb[:], cw_d.ap().rearrange("(b p) k -> p b k", p=128)
            )
            cb_sb = consts.tile([128, nH], F32, tag="cb")
            nc.sync.dma_start(cb_sb[:], cb_d.ap().rearrange("(b p) -> p b", p=128))
            cvec_sb = consts.tile([128, nH], F32, tag="cvec")
            nc.sync.dma_start(
                cvec_sb[:], cvec_d.ap().rearrange("(b p) -> p b", p=128)
            )
            bg_sb = consts.tile([128, nG], F32, tag="bg")
            nc.sync.dma_start(bg_sb[:], bg_d.ap().rearrange("(b p) -> p b", p=128))
            wmask_sb = consts.tile([128, 1], F32, tag="wmask")
            nc.sync.dma_start(wmask_sb[:], wmask_d.ap().rearrange("(p o) -> p o", o=1))
            hist = consts.tile([128, nH * 3], BF16, tag="hist")
            nc.vector.memset(hist[:], 0.0)
            carry = consts.tile([128, nH], F32, tag="carry")
            nc.vector.memset(carry[:], 0.0)
            zero1 = consts.tile([128, 1], F32, tag="zero1")
            nc.vector.memset(zero1[:], 0.0)
            onep = consts.tile([128, 1], F32, tag="onep")
            nc.vector.memset(onep[:], 1.0 + EPS)

            # Weight pools for all phases, loaded up front so the DMAs
            # overlap phase A compute instead of stalling phase boundaries.
            wg_pool = tc.tile_pool(name="wg", bufs=1)
            wg = wg_pool.__enter__()
            wg_sb = []
            for hb in range(nH):
                t = wg.tile([128, 2 * H], BF16, tag=f"wg{hb}")
                nc.sync.dma_start(t[:], wgT_d[hb * 128 : (hb + 1) * 128, :])
                wg_sb.append(t)
            wo_pool = tc.tile_pool(name="wo", bufs=1)
            wo = wo_pool.__enter__()
            wo_sb = []
            for hb in range(nH):
                t = wo.tile([128, D], BF16, tag=f"wo{hb}")
                nc.sync.dma_start(t[:], woutT_d[hb * 128 : (hb + 1) * 128, :])
                wo_sb.append(t)
            pcz_pool = tc.tile_pool(name="pc_z", bufs=12)
            pc_z = pcz_pool.__enter__()
            pcot_pool = tc.tile_pool(name="pc_ot", bufs=2)
            pc_ot = pcot_pool.__enter__()
            psoc_pool = tc.tile_pool(name="ps_oc", bufs=2, space="PSUM")
            ps_oc = psoc_pool.__enter__()

            # ================= PHASE A =================
            with (
                tc.tile_pool(name="wa", bufs=1) as wa,
                tc.tile_pool(name="pa", bufs=5) as pa_xin,
                tc.tile_pool(name="pa_xT", bufs=10) as pa_xT,
                tc.tile_pool(name="pa_ext", bufs=3) as pa_ext,
                tc.tile_pool(name="pa_xb", bufs=6) as pa_xb,
                tc.tile_pool(name="pa_g", bufs=3) as pa_g,
                tc.tile_pool(name="ps_tp", bufs=2, space="PSUM") as ps_tp,
                tc.tile_pool(name="ps_gx", bufs=4, space="PSUM") as ps_gx,
            ):
                win_sb = []
                for d in range(nD):
                    t = wa.tile([128, 2 * H], BF16, tag=f"win{d}")
                    nc.sync.dma_start(t[:], winT_d[d * 128 : (d + 1) * 128, :])
                    win_sb.append(t)

                for c0, cw, warm in tiles_of(TBA):
                    ntp = cw // 128
                    xts = []
                    for i in range(ntp):
                        xt = pa_xin.tile([128, D], F32, tag="xin")
                        nc.sync.dma_start(
                            xt[:], xin_d[c0 + i * 128 : c0 + (i + 1) * 128, :]
                        )
                        xts.append(xt)
                    xT = []
                    for d in range(nD):
                        ps = ps_tp.tile([128, cw], F32, tag="tp")
                        for i in range(ntp):
                            nc.tensor.transpose(
                                ps[:, i * 128 : (i + 1) * 128],
                                xts[i][:, d * 128 : (d + 1) * 128],
                                ident[:],
                            )
                        xTd = pa_xT.tile([128, cw], BF16, tag="xT")
                        nc.scalar.copy(xTd[:], ps[:])
                        xT.append(xTd)

                    # g-blocks 0..11 (gate rows): gelu, grouped on ACT
                    if not warm:
                        for g in range(nH):
                            ps = ps_gx.tile([128, cw], F32, tag="gx")
                            for d in range(nD):
                                nc.tensor.matmul(
                                    ps[:],
                                    win_sb[d][:, g * 128 : (g + 1) * 128],
                                    xT[d][:],
                                    start=(d == 0), stop=(d == nD - 1),
                                )
                            gg = pa_g.tile([128, cw], BF16, tag="gg")
                            nc.scalar.activation(
                                gg[:], ps[:], AF.Gelu, bias=zero1[:, 0:1]
                            )
                            nc.sync.dma_start(
                                gate_s[g, :, c0 - W : c0 - W + cw], gg[:]
                            )
                    # g-blocks 12..23 (xb rows): depthwise causal conv
                    for g in range(nH, nG):
                        ps = ps_gx.tile([128, cw], F32, tag="gx")
                        for d in range(nD):
                            nc.tensor.matmul(
                                ps[:],
                                win_sb[d][:, g * 128 : (g + 1) * 128],
                                xT[d][:],
                                start=(d == 0), stop=(d == nD - 1),
                            )
                        b = g - nH
                        ext = pa_ext.tile([128, TBA + 3], BF16, tag="ext")
                        nc.vector.tensor_copy(
                            ext[:, 0:3], hist[:, b * 3 : b * 3 + 3]
                        )
                        nc.vector.tensor_copy(ext[:, 3 : 3 + cw], ps[:])
                        nc.vector.tensor_copy(
                            hist[:, b * 3 : b * 3 + 3], ext[:, cw : cw + 3]
                        )
                        x0 = pa_xb.tile([128, cw], BF16, tag="xbt")
                        nc.vector.tensor_scalar(
                            x0[:], ext[:, 3 : 3 + cw],
                            cw_sb[:, b, 3:4], cb_sb[:, b : b + 1],
                            ALU.mult, ALU.add,
                        )
                        for k in (2, 1, 0):
                            x1 = pa_xb.tile([128, cw], BF16, tag="xbt")
                            nc.vector.scalar_tensor_tensor(
                                x1[:], ext[:, k : k + cw],
                                cw_sb[:, b, k : k + 1], x0[:],
                                ALU.mult, ALU.add,
                            )
                            x0 = x1
                        nc.sync.dma_start(xb_s[b, :, c0 : c0 + cw], x0[:])

            # ================= PHASE B =================
            with (
                tc.tile_pool(name="pb_xb", bufs=18) as pb_xb,
                tc.tile_pool(name="pb_sf", bufs=13) as pb_sf,
                tc.tile_pool(name="pb_si", bufs=13) as pb_si,
                tc.tile_pool(name="pb_al", bufs=11) as pb_al,
                tc.tile_pool(name="pb_be", bufs=4) as pb_be,
                tc.tile_pool(name="pb_a2", bufs=2) as pb_a2,
                tc.tile_pool(name="pb_xs", bufs=3) as pb_xs,
                tc.tile_pool(name="pb_h", bufs=3) as pb_h,
                tc.tile_pool(name="pb_z", bufs=2) as pb_z,
                tc.tile_pool(name="pb_gi", bufs=2) as pb_gi,
                tc.tile_pool(name="ps_fg", bufs=6, space="PSUM") as ps_fg,
            ):
                for c0, cw, warm in tiles_of(TBB):
                    xbin = []
                    for hb in range(nH):
                        t = pb_xb.tile([128, TBB], BF16, tag="xbin")
                        nc.sync.dma_start(t[:, :cw], xb_s[hb, :, c0 : c0 + cw])
                        xbin.append(t)
                    # pass 1: all fg matmuls + sigmoids (one ACT function)
                    sf, si = [], []
                    for b in range(nH):
                        for part in (0, nH):
                            g = part + b
                            ps = ps_fg.tile([128, cw], F32, tag="fg")
                            for hb in range(nH):
                                nc.tensor.matmul(
                                    ps[:],
                                    wg_sb[hb][:, g * 128 : (g + 1) * 128],
                                    xbin[hb][:, :cw],
                                    start=(hb == 0), stop=(hb == nH - 1),
                                )
                            if part == 0:
                                t = pb_sf.tile([128, cw], BF16, tag="sf")
                                sf.append(t)
                            else:
                                t = pb_si.tile([128, cw], BF16, tag="si")
                                si.append(t)
                            nc.scalar.activation(
                                t[:], ps[:], AF.Sigmoid,
                                bias=bg_sb[:, g : g + 1],
                            )
                    # pass 2: alpha = exp(cvec * sigmoid(f))  (one function)
                    al = []
                    for b in range(nH):
                        t = pb_al.tile([128, cw], F32, tag="al")
                        nc.scalar.activation(
                            t[:], sf[b][:], AF.Exp, bias=zero1[:, 0:1],
                            scale=cvec_sb[:, b : b + 1],
                        )
                        al.append(t)
                    # pass 3: beta + xs + scan + z per block (sqrt ops stay
                    # contiguous on ACT; everything else is DVE)
                    for b in range(nH):
                        a2 = pb_a2.tile([128, cw], F32, tag="a2")
                        nc.vector.tensor_mul(a2[:], al[b][:], al[b][:])
                        be = pb_be.tile([128, cw], BF16, tag="be")
                        nc.scalar.activation(
                            be[:], a2[:], AF.Sqrt, bias=onep[:, 0:1], scale=-1.0
                        )
                        xs = pb_xs.tile([128, cw], F32, tag="xs")
                        nc.vector.tensor_mul(
                            xs[:], si[b][:], xbin[b][:, :cw]
                        )
                        xs2 = pb_xs.tile([128, cw], F32, tag="xs")
                        nc.vector.tensor_mul(xs2[:], xs[:], be[:])
                        if warm:
                            xs3 = pb_xs.tile([128, cw], F32, tag="xs")
                            nc.vector.tensor_scalar_mul(
                                xs3[:], xs2[:], wmask_sb[:, 0:1]
                            )
                            xs2 = xs3
                        h = pb_h.tile([128, cw], F32, tag="h")
                        nc.vector.tensor_tensor_scan(
                            h[:], al[b][:], xs2[:], carry[:, b : b + 1],
                            ALU.mult, ALU.add,
                        )
                        nc.vector.tensor_copy(
                            carry[:, b : b + 1], h[:, cw - 1 : cw]
                        )
                        if not warm:
                            gi = pb_gi.tile([128, cw], BF16, tag="gi")
                            nc.sync.dma_start(
                                gi[:], gate_s[b, :, c0 - W : c0 - W + cw]
                            )
                            z = pb_z.tile([128, cw], BF16, tag="z")
                            nc.vector.tensor_mul(z[:], h[:], gi[:])
                            nc.sync.dma_start(
                                z_s[b, :, c0 - W : c0 - W + cw], z[:]
                            )

            # ================= PHASE C =================
            # out[t, d] = sum_h z[h, t] * W_outT[h, d]: use the z t-chunk as
            # the stationary operand so the result lands directly in [t, d]
            # layout -- no PE transposes or extra copies needed.
            for k in range(T_loc // TBC):
                c0, cw = k * TBC, TBC
                zin = []
                for hb in range(nH):
                    t = pc_z.tile([128, TBC], BF16, tag="zin")
                    nc.sync.dma_start(t[:], z_s[hb, :, c0 : c0 + cw])
                    zin.append(t)
                for tq in range(cw // 128):
                    pss = []
                    for dh in range(2):
                        ps = ps_oc.tile([128, 512], F32, tag="oc")
                        for hb in range(nH):
                            nc.tensor.matmul(
                                ps[:],
                                zin[hb][:, tq * 128 : (tq + 1) * 128],
                                wo_sb[hb][:, dh * 512 : (dh + 1) * 512],
                                start=(hb == 0), stop=(hb == nH - 1),
                            )
                        pss.append(ps)
                    otile = pc_ot.tile([128, D], F32, tag="otile")
                    nc.scalar.copy(otile[:, 0:512], pss[0][:])
                    nc.scalar.copy(otile[:, 512:1024], pss[1][:])
                    nc.sync.dma_start(
                        out_d[c0 + tq * 128 : c0 + (tq + 1) * 128, :],
                        otile[:],
                    )

            psoc_pool.__exit__(None, None, None)
            pcot_pool.__exit__(None, None, None)
            pcz_pool.__exit__(None, None, None)
            wo_pool.__exit__(None, None, None)
            wg_pool.__exit__(None, None, None)

    nc.compile()
    return nc


def _prep_shared(W_in, conv_w, conv_b, W_g, b_g, forget_base, W_out):
    sp = np.log1p(np.exp(forget_base.astype(np.float64))).astype(np.float32)
    b16 = lambda a: np.ascontiguousarray(a).astype(ml_dtypes.bfloat16)
    return {
        "winT": b16(W_in.T),
        "wgT": b16(W_g.T),
        "woutT": b16(W_out.T),
        "cw": np.ascontiguousarray(conv_w[:, 0, :]),
        "cb": np.ascontiguousarray(conv_b),
        "cvec": np.ascontiguousarray(-8.0 * sp),
        "bg": np.ascontiguousarray(b_g),
        "ident": np.eye(128, dtype=np.float32),
    }


def run_sharded(inputs, T_loc=2048, W=128, TBA=512, TBB=512, TBC=512,
                nc=None, profile_hook=None):
    x = inputs["x"]
    N, T, D = x.shape
    H = inputs["W_g"].shape[1]
    assert T == 2 * T_loc
    if nc is None:
        nc = build_nc(T_loc=T_loc, W=W, TBA=TBA, TBB=TBB, TBC=TBC, D=D, H=H)
    shared = _prep_shared(
        inputs["W_in"], inputs["conv_w"], inputs["conv_b"], inputs["W_g"],
        inputs["b_g"], inputs["forget_base"], inputs["W_out"],
    )
    in_maps = []
    for core in range(8):
        n, half = core // 2, core % 2
        t0 = half * T_loc
        xin = np.zeros((W + T_loc, D), np.float32)
        lo = max(0, t0 - W)
        xin[W - (t0 - lo):] = x[n, lo : t0 + T_loc]
        m = dict(shared)
        m["xin"] = xin
        m["wmask"] = np.full((128,), float(half), np.float32)
        in_maps.append(m)
    if profile_hook is not None:
        with profile_hook():
            res = run_bass_kernel_spmd(nc, in_maps, core_ids=list(range(8)))
    else:
        res = run_bass_kernel_spmd(nc, in_maps, core_ids=list(range(8)))
    out = np.empty((N, T, D), np.float32)
    for core in range(8):
        n, half = core // 2, core % 2
        out[n, half * T_loc : (half + 1) * T_loc] = res.results[core]["out"]
    return out


def kernel(**inputs):
    return run_sharded(inputs)
